# revision 40
# baseline (speedup 1.0000x reference)
"""Trainium2 Bass kernel for AdvancedDualTargetPredictor (cross-attention
transformer block).

Sharding: pure data-parallel over batch B=8 across the 8 NeuronCores.
Each core runs one batch element end-to-end; no collectives.

Per-core dataflow (token dim q/k = 1024, model dim D = 512, H = 8 heads,
head dim 64, FFN = 2048):
  1. PE-transpose drug/prot -> drugT/protT  [D, tok] layout
  2. QT = wq.T @ drugT, KT = wk.T @ protT   [D, tok]
     V  = protT.T @ wv                       [tok, D] (natural), augmented
          with a ones column per head for softmax row sums
  3. per head: scoresT = K @ Q.T  (PSUM), exp via ACT (scale=1/8 fused),
     ctx^T/sums = [V_h | 1].T @ ET  accumulated over k chunks, copied to
     SBUF unnormalized; after the loop all heads are normalized in a
     batch: reciprocal of the sums row, K=1 PE matmul broadcast across
     64 partitions, in-place scale.  (No DMA in the critical chain and
     double-buffered PSUM everywhere, so the PE never idles long enough
     for the HAM clock throttle to kick in.)
  4. attn_out = ctxT.T @ wo (+bo), residual + LN1 (bn_stats, natural layout)
  5. PE-transpose x -> xT; h1T = w1.T @ xT with fused gelu(+b1) on ACT
     (w1 is prefetched during attention)
  6. ffn = h1T.T @ w2 (+b2), residual + LN2 -> out
All matmuls run as float32r (full PE rate for free dim >= 256); every
producer feeding a matmul writes float32r (rounded) per the BIR verifier.
Pools are allocated/released per phase and per SBUF side: reservations are
static per pool lifetime, so phase scoping keeps peak usage in budget.
"""

import numpy as np

B, NQ, NK, D, H = 8, 1024, 1024, 512, 8
HD = D // H  # 64
FFN = 4 * D  # 2048
P = 128
KD = D // P  # 4 chunks of the model dim
QM = NQ // P  # 8 token chunks
FM = FFN // P  # 16 ffn chunks
SCALE = HD ** -0.5
EPS = 1e-5

INPUT_NAMES = [
    "drug", "prot", "wq", "bq", "wk", "bk", "wv", "bv", "wo", "bo",
    "ln1_g", "ln1_b", "ln2_g", "ln2_b", "w1", "b1", "w2", "b2",
]

_CACHE = {}


def _build(flags, use_f32r=True, act_name="Gelu_apprx_tanh"):
    import concourse.bass as bass
    import concourse.bacc as bacc
    import concourse.mybir as mybir
    import concourse.tile as tile
    from concourse.masks import make_identity

    f32 = mybir.dt.float32
    mmdt = mybir.dt.float32r if use_f32r else f32
    AF = mybir.ActivationFunctionType
    OP = mybir.AluOpType

    (has_bq, has_bk, has_bv, has_bo, has_b1, has_b2,
     has_g1, has_be1, has_g2, has_be2) = flags

    nc = bacc.Bacc(None)

    dr = {}
    shapes = {
        "drug": [NQ, D], "prot": [NK, D],
        "wq": [D, D], "wk": [D, D], "wv": [D, D], "wo": [D, D],
        "bq": [D], "bk": [D], "bv": [D], "bo": [D],
        "ln1_g": [D], "ln1_b": [D], "ln2_g": [D], "ln2_b": [D],
        "w1": [D, FFN], "b1": [FFN], "w2": [FFN, D], "b2": [D],
    }
    mm_input_names = ("wq", "wk", "wv", "wo", "w1", "w2")
    for name in INPUT_NAMES:
        dt_in = mmdt if name in mm_input_names else f32
        dr[name] = nc.dram_tensor(name, shapes[name], dt_in, kind="ExternalInput")
    out_dram = nc.dram_tensor("out", [NQ, D], f32, kind="ExternalOutput")

    def bcast_dram(ap1d, parts):
        # DMA-broadcast a 1-D DRAM vector across `parts` partitions
        return bass.AP(tensor=ap1d.tensor, offset=ap1d.offset,
                       ap=[[0, parts]] + [list(x) for x in ap1d.ap])

    with tile.TileContext(nc) as tc:
        pool = lambda nm, n=1, space="SBUF", side=None: tc.alloc_tile_pool(
            name=nm, bufs=n, space=space, side=side)

        # ---------- constants (whole kernel) ----------
        cn = pool("cn", side="left")
        ident = cn.tile([P, P], f32)
        make_identity(nc, ident)

        bq_col = bk_col = bv_bc = bo_bc = b1_col = b2_bc = None
        g1_bc = be1_bc = g2_bc = be2_bc = None
        if has_bq:
            bq_col = cn.tile([P, KD], f32, tag="bq")
            nc.sync.dma_start(bq_col, dr["bq"][:].rearrange("(ko p) -> p ko", p=P))
        if has_bk:
            bk_col = cn.tile([P, KD], f32, tag="bk")
            nc.sync.dma_start(bk_col, dr["bk"][:].rearrange("(ko p) -> p ko", p=P))
        if has_bv:
            bv_bc = cn.tile([P, D], f32, tag="bv")
            nc.sync.dma_start(bv_bc, bcast_dram(dr["bv"][:], P))
        if has_bo:
            bo_bc = cn.tile([P, D], f32, tag="bo")
            nc.sync.dma_start(bo_bc, bcast_dram(dr["bo"][:], P))
        if has_b2:
            b2_bc = cn.tile([P, D], f32, tag="b2")
            nc.sync.dma_start(b2_bc, bcast_dram(dr["b2"][:], P))
        if has_b1:
            b1_col = cn.tile([P, FM], f32, tag="b1")
            nc.sync.dma_start(b1_col, dr["b1"][:].rearrange("(ko p) -> p ko", p=P))
        if has_g1:
            g1_bc = cn.tile([P, D], f32, tag="g1")
            nc.sync.dma_start(g1_bc, bcast_dram(dr["ln1_g"][:], P))
        if has_be1:
            be1_bc = cn.tile([P, D], f32, tag="be1")
            nc.sync.dma_start(be1_bc, bcast_dram(dr["ln1_b"][:], P))
        if has_g2:
            g2_bc = cn.tile([P, D], f32, tag="g2")
            nc.sync.dma_start(g2_bc, bcast_dram(dr["ln2_g"][:], P))
        if has_be2:
            be2_bc = cn.tile([P, D], f32, tag="be2")
            nc.sync.dma_start(be2_bc, bcast_dram(dr["ln2_b"][:], P))

        eps_t = cn.tile([P, 1], f32, tag="eps")
        nc.vector.memset(eps_t, EPS)
        zero_t = cn.tile([P, 1], f32, tag="zero")
        nc.vector.memset(zero_t, 0.0)
        ones_h = cn.tile([P, H, 1], f32, tag="ones_h")
        nc.vector.memset(ones_h, 1.0)
        # ones row at partition 64, f32r, lhsT of the K=1 broadcast matmul
        onesf = cn.tile([HD + 1, HD], f32, tag="onesf")
        nc.vector.memset(onesf, 1.0)
        ones_r = cn.tile([HD + 1, HD], mmdt, tag="ones_r")
        nc.vector.tensor_copy(ones_r, onesf)
        ident_r = cn.tile([P, P], mmdt, tag="ident_r")
        nc.vector.tensor_copy(ident_r, ident)
        warm_f = cn.tile([P, 512], f32, tag="warm_f")
        nc.vector.memset(warm_f, 0.5)
        warm_src = cn.tile([P, 512], mmdt, tag="warm_src")
        nc.vector.tensor_copy(warm_src, warm_f)

        # tiny long-lived stats tiles
        mv1 = cn.tile([P, QM, 2], f32, tag="mv1")
        mv2 = cn.tile([P, QM, 2], f32, tag="mv2")
        lt1 = cn.tile([P, QM], f32, tag="lt1")
        rstd1 = cn.tile([P, QM], f32, tag="rstd1")
        lt2 = cn.tile([P, QM], f32, tag="lt2")
        rstd2 = cn.tile([P, QM], f32, tag="rstd2")

        def warm_burst(ps_pool, tag, n_mm, bufs=1):
            # dependency-free back-to-back matmuls; >=3.4us of contiguous PE
            # activity releases the HAM clock throttle (K=4/8 -> 8/8)
            wp = ps_pool.tile([P, 512], f32, tag=tag, bufs=bufs)
            for _ in range(n_mm):
                nc.tensor.matmul(wp, lhsT=ident_r, rhs=warm_src,
                                 start=True, stop=True)

        # ---------- phase 1: load + transpose inputs ----------
        pAB = pool("pAB", side="right")
        psA = pool("psA", space="PSUM")  # touch(1) + tp(2) + mm(3) = 6 banks

        drug_nat = pAB.tile([P, QM, D], f32, tag="dn")
        prot_nat = pAB.tile([P, QM, D], f32, tag="pn")
        nc.sync.dma_start(drug_nat, dr["drug"][:].rearrange("(m p) d -> p m d", p=P))
        nc.sync.dma_start(prot_nat, dr["prot"][:].rearrange("(m p) d -> p m d", p=P))
        # QKV weights live with the inputs (all dead after phase 2)
        wq_sb = pAB.tile([P, KD, D], mmdt, tag="wq")
        wk_sb = pAB.tile([P, KD, D], mmdt, tag="wk")
        wv_sb = pAB.tile([P, KD, D], mmdt, tag="wv")
        for w_sb, name in ((wq_sb, "wq"), (wk_sb, "wk"), (wv_sb, "wv")):
            nc.sync.dma_start(
                w_sb, dr[name][:].rearrange("(ko p) n -> p ko n", p=P))

        # absorber matmuls: a transpose's LDW struct only fits ONE sync wait
        # (walrus limit), so let a normal matmul observe each input DMA (and
        # the ident producer) on PE before the transposes run.
        warm_burst(psA, "touch", 10, bufs=1)
        touch = psA.tile([P, 1], f32, tag="touch", bufs=1)
        nc.tensor.matmul(touch, lhsT=ident, rhs=drug_nat[:, 0, 0:1],
                         start=True, stop=True)
        nc.tensor.matmul(touch, lhsT=ident, rhs=prot_nat[:, 0, 0:1],
                         start=True, stop=True)

        drugT = pAB.tile([P, KD, NQ], mmdt, tag="dT")
        protT = pAB.tile([P, KD, NK], mmdt, tag="pT")
        for src, dst in ((drug_nat, drugT), (prot_nat, protT)):
            for m in range(QM):
                for c in range(KD):
                    pt = psA.tile([P, P], f32, tag="tp", bufs=2)
                    nc.tensor.transpose(pt, src[:, m, c * P:(c + 1) * P], ident)
                    nc.vector.tensor_copy(dst[:, c, m * P:(m + 1) * P], pt)

        # ---------- phase 2: QKV projections ----------
        warm_burst(psA, "touch", 10, bufs=1)
        pC = pool("pC", side="left")
        # wo stored per-head at base partition 0 to match ctxT tiles
        wo_sb = pC.tile([HD, H, D], mmdt, tag="wo")
        nc.sync.dma_start(
            wo_sb, dr["wo"][:].rearrange("(h p) n -> p h n", p=HD))
        QT = pC.tile([P, KD, NQ], mmdt, tag="QT")
        KT = pC.tile([P, KD, NK], mmdt, tag="KT")
        for (w_sb, src, dst, bias) in (
            (wq_sb, drugT, QT, bq_col),
            (wk_sb, protT, KT, bk_col),
        ):
            for mo in range(KD):
                for qc in range(2):
                    pq = psA.tile([P, 512], f32, tag="mm", bufs=3)
                    for kd in range(KD):
                        nc.tensor.matmul(
                            pq,
                            lhsT=w_sb[:, kd, mo * P:(mo + 1) * P],
                            rhs=src[:, kd, qc * 512:(qc + 1) * 512],
                            start=(kd == 0), stop=(kd == KD - 1))
                    o = dst[:, mo, qc * 512:(qc + 1) * 512]
                    if bias is not None:
                        nc.vector.tensor_scalar_add(o, pq, bias[:, mo:mo + 1])
                    else:
                        nc.vector.tensor_copy(o, pq)

        # V in natural layout, augmented with ones column per head
        Vaug = pC.tile([P, QM, H, HD + 1], mmdt, tag="Va")
        for m in range(QM):
            pv = psA.tile([P, 512], f32, tag="mm", bufs=3)
            for kd in range(KD):
                nc.tensor.matmul(
                    pv,
                    lhsT=protT[:, kd, m * P:(m + 1) * P],
                    rhs=wv_sb[:, kd, :],
                    start=(kd == 0), stop=(kd == KD - 1))
            o = Vaug[:, m, :, 0:HD]
            pv_v = pv.rearrange("p (h d) -> p h d", h=H)
            if has_bv:
                nc.vector.tensor_add(o, pv_v, bv_bc.rearrange("p (h d) -> p h d", h=H))
            else:
                nc.vector.tensor_copy(o, pv_v)
            nc.vector.tensor_copy(Vaug[:, m, :, HD:HD + 1], ones_h)

        pAB.release()
        psA.release()

        # ---------- phase 3: attention ----------
        pW = pool("pW", side="right")   # w1 prefetched during attention
        w1_sb = pW.tile([P, KD, FFN], mmdt, tag="w1")
        nc.sync.dma_start(w1_sb, dr["w1"][:].rearrange("(ko p) n -> p ko n", p=P))

        pD = pool("pD", side="right")
        pE = pool("pE", side="left")
        psB = pool("psB", space="PSUM")  # sc(2x2) + cx(2x2) = 8 banks

        warm_burst(psB, "sc", 10, bufs=2)

        ctxUs = []
        for h in range(H):
            kt_c = h // 2
            roff = HD * (h % 2)
            cx = psB.tile([HD + 1, 2, 512], f32, tag="cx", bufs=1)
            for kc in range(QM):
                sc_ps = psB.tile([P, 1024], f32, tag="sc", bufs=2)
                for qc in range(2):
                    nc.tensor.matmul(
                        sc_ps[:, qc * 512:(qc + 1) * 512],
                        lhsT=KT[roff:roff + HD, kt_c, kc * P:(kc + 1) * P],
                        rhs=QT[roff:roff + HD, kt_c, qc * 512:(qc + 1) * 512],
                        start=True, stop=True)
                et = pD.tile([P, 1024], mmdt, tag="et", bufs=3)
                nc.scalar.activation(et, sc_ps, AF.Exp, scale=SCALE)
                for qc in range(2):
                    nc.tensor.matmul(
                        cx[:, qc, :],
                        lhsT=Vaug[:, kc, h, :],
                        rhs=et[:, qc * 512:(qc + 1) * 512],
                        start=(kc == 0), stop=(kc == QM - 1))
            # move unnormalized ctx + sums row to SBUF; frees the PSUM slot
            ctxU = pE.tile([HD + 1, 2, 512], mmdt, tag="ctxU", bufs=H)
            nc.vector.tensor_copy(ctxU, cx)
            ctxUs.append(ctxU)
            # per-head softmax normalization, pipelined with the next head's
            # score matmuls: recip sums in place, K=1 PE matmul broadcasts
            # them across the 64 partitions, in-place scale.
            with nc.allow_low_precision(reason="f32r softmax denominators"):
                nc.vector.reciprocal(ctxU[HD:HD + 1, :, :],
                                     ctxU[HD:HD + 1, :, :])
            for qc in range(2):
                rbp = psB.tile([HD, 512], f32, tag="rb", bufs=2)
                nc.tensor.matmul(rbp, lhsT=ones_r[HD:HD + 1, :],
                                 rhs=ctxU[HD:HD + 1, qc, :],
                                 start=True, stop=True)
                rbs = pD.tile([HD, 512], f32, tag="rbs", bufs=2)
                nc.vector.tensor_copy(rbs, rbp)
                nc.vector.tensor_mul(ctxU[0:HD, qc, :], ctxU[0:HD, qc, :], rbs)

        pD.release()
        psB.release()

        # ---------- phase 4: attn_out + residual + LN1 ----------
        pF = pool("pF", side="right")
        psC = pool("psC", space="PSUM")  # mm(2) + tp(2) + f1(2x2) = 8 banks

        warm_burst(psC, "mm", 10, bufs=2)
        drug2 = pF.tile([P, QM, D], f32, tag="dn2")
        nc.sync.dma_start(drug2, dr["drug"][:].rearrange("(m p) d -> p m d", p=P))
        x_nat = pF.tile([P, QM, D], f32, tag="xn")
        for qm in range(QM):
            pa = psC.tile([P, 512], f32, tag="mm", bufs=2)
            for h in range(H):
                nc.tensor.matmul(
                    pa,
                    lhsT=ctxUs[h][0:HD, :, :].rearrange("p a b -> p (a b)")[
                        :, qm * P:(qm + 1) * P],
                    rhs=wo_sb[:, h, :],
                    start=(h == 0), stop=(h == H - 1))
            t = x_nat[:, qm, :]
            nc.vector.tensor_add(t, pa, drug2[:, qm, :])
            if has_bo:
                nc.vector.tensor_add(t, t, bo_bc)
            st = pF.tile([P, nc.vector.BN_STATS_DIM], f32, tag="st", bufs=2)
            nc.vector.bn_stats(st, t)
            nc.vector.bn_aggr(mv1[:, qm, :], st)
            nc.scalar.activation(lt1[:, qm:qm + 1], mv1[:, qm, 1:2],
                                 AF.Ln, bias=eps_t)
            nc.scalar.activation(rstd1[:, qm:qm + 1], lt1[:, qm:qm + 1],
                                 AF.Exp, scale=-0.5)
            nc.vector.tensor_scalar(t, t, mv1[:, qm, 0:1], rstd1[:, qm:qm + 1],
                                    OP.subtract, OP.mult)
            if has_g1:
                nc.vector.tensor_mul(t, t, g1_bc)
            if has_be1:
                nc.vector.tensor_add(t, t, be1_bc)

        pE.release()  # ctxU dead
        pC.release()  # QT/KT/Vaug/wo dead

        # ---------- phase 5+6: x transpose + FFN ----------
        pG = pool("pG", side="left")
        w2_sb = pG.tile([P, FM, D], mmdt, tag="w2")
        nc.sync.dma_start(w2_sb, dr["w2"][:].rearrange("(ko p) n -> p ko n", p=P))

        xT = pG.tile([P, KD, NQ], mmdt, tag="xT")
        for m in range(QM):
            for c in range(KD):
                pt = psC.tile([P, P], f32, tag="tp", bufs=2)
                nc.tensor.transpose(pt, x_nat[:, m, c * P:(c + 1) * P], ident)
                nc.vector.tensor_copy(xT[:, c, m * P:(m + 1) * P], pt)

        out_v = out_dram[:].rearrange("(m p) d -> p m d", p=P)
        x2 = pF.tile([P, QM, D], f32, tag="dn2")  # reuse drug2's slot
        for qh in range(2):  # FFN in two query halves to halve h1 residency
            h1T = pG.tile([P, FM, 512], mmdt, tag="h1", bufs=1)
            for mo in range(FM):
                pf = psC.tile([P, 512], f32, tag="f1", bufs=2)
                for kd in range(KD):
                    nc.tensor.matmul(
                        pf,
                        lhsT=w1_sb[:, kd, mo * P:(mo + 1) * P],
                        rhs=xT[:, kd, qh * 512:(qh + 1) * 512],
                        start=(kd == 0), stop=(kd == KD - 1))
                nc.scalar.activation(
                    h1T[:, mo, :], pf, getattr(AF, act_name),
                    bias=(b1_col[:, mo:mo + 1] if has_b1 else zero_t))
            for qj in range(4):
                qm = qh * 4 + qj
                pf2 = psC.tile([P, 512], f32, tag="mm", bufs=2)
                for kc in range(FM):
                    nc.tensor.matmul(
                        pf2,
                        lhsT=h1T[:, kc, qj * P:(qj + 1) * P],
                        rhs=w2_sb[:, kc, :],
                        start=(kc == 0), stop=(kc == FM - 1))
                t = x2[:, qm, :]
                nc.vector.tensor_add(t, pf2, x_nat[:, qm, :])
                if has_b2:
                    nc.vector.tensor_add(t, t, b2_bc)
                st = pF.tile([P, nc.vector.BN_STATS_DIM], f32, tag="st", bufs=2)
                nc.vector.bn_stats(st, t)
                nc.vector.bn_aggr(mv2[:, qm, :], st)
                nc.scalar.activation(lt2[:, qm:qm + 1], mv2[:, qm, 1:2],
                                     AF.Ln, bias=eps_t)
                nc.scalar.activation(rstd2[:, qm:qm + 1], lt2[:, qm:qm + 1],
                                     AF.Exp, scale=-0.5)
                ob = pG.tile([P, D], f32, tag="ob", bufs=3)
                nc.vector.tensor_scalar(ob, t, mv2[:, qm, 0:1],
                                        rstd2[:, qm:qm + 1],
                                        OP.subtract, OP.mult)
                if has_g2:
                    nc.vector.tensor_mul(ob, ob, g2_bc)
                if has_be2:
                    nc.vector.tensor_add(ob, ob, be2_bc)
                nc.sync.dma_start(out_v[:, qm, :], ob)

        pF.release()
        pW.release()
        pG.release()
        psC.release()
        cn.release()  # left stack: pG popped first, then cn

    nc.finalize()
    return nc


def _flags_from_inputs(inputs):
    def nz(name):
        return bool(np.any(inputs[name] != 0.0))

    return (
        nz("bq"), nz("bk"), nz("bv"), nz("bo"), nz("b1"), nz("b2"),
        bool(np.any(inputs["ln1_g"] != 1.0)), nz("ln1_b"),
        bool(np.any(inputs["ln2_g"] != 1.0)), nz("ln2_b"),
    )


def build_nc(inputs, use_f32r=True, act_name="Gelu_apprx_tanh"):
    flags = _flags_from_inputs(inputs)
    key = (flags, use_f32r, act_name)
    if key not in _CACHE:
        _CACHE[key] = _build(flags, use_f32r=use_f32r, act_name=act_name)
    return _CACHE[key]


def kernel(**inputs):
    from concourse.bass_utils import run_bass_kernel_spmd

    inputs = {k: np.ascontiguousarray(np.asarray(v, dtype=np.float32))
              for k, v in inputs.items()}
    nc = build_nc(inputs)
    in_maps = []
    for b in range(B):
        m = {}
        for name in INPUT_NAMES:
            v = inputs[name]
            m[name] = v[b] if name in ("drug", "prot") else v
        in_maps.append(m)
    res = run_bass_kernel_spmd(nc, in_maps, list(range(B)))
    out = np.stack([res.results[i]["out"] for i in range(B)], axis=0)
    return out.astype(np.float32)


# revision 41
# speedup vs baseline: 1.1914x; 1.1914x over previous
"""Trainium2 Bass kernel for AdvancedDualTargetPredictor (cross-attention
transformer block).

Sharding: pure data-parallel over batch B=8 across the 8 NeuronCores.
Each core runs one batch element end-to-end; no collectives.

Per-core dataflow (token dim q/k = 1024, model dim D = 512, H = 8 heads,
head dim 64, FFN = 2048):
  1. PE-transpose drug/prot -> drugT/protT  [D, tok] layout
  2. QT = wq.T @ drugT, KT = wk.T @ protT   [D, tok]
     V  = protT.T @ wv                       [tok, D] (natural), augmented
          with a ones column per head for softmax row sums
  3. per head: scoresT = K @ Q.T  (PSUM), exp via ACT (scale=1/8 fused),
     ctx^T/sums = [V_h | 1].T @ ET  accumulated over k chunks, copied to
     SBUF unnormalized; after the loop all heads are normalized in a
     batch: reciprocal of the sums row, K=1 PE matmul broadcast across
     64 partitions, in-place scale.  (No DMA in the critical chain and
     double-buffered PSUM everywhere, so the PE never idles long enough
     for the HAM clock throttle to kick in.)
  4. attn_out = ctxT.T @ wo (+bo), residual + LN1 (bn_stats, natural layout)
  5. PE-transpose x -> xT; h1T = w1.T @ xT with fused gelu(+b1) on ACT
     (w1 is prefetched during attention)
  6. ffn = h1T.T @ w2 (+b2), residual + LN2 -> out
All matmuls run as float32r (full PE rate for free dim >= 256); every
producer feeding a matmul writes float32r (rounded) per the BIR verifier.
Pools are allocated/released per phase and per SBUF side: reservations are
static per pool lifetime, so phase scoping keeps peak usage in budget.
"""

import numpy as np

B, NQ, NK, D, H = 8, 1024, 1024, 512, 8
HD = D // H  # 64
FFN = 4 * D  # 2048
P = 128
KD = D // P  # 4 chunks of the model dim
QM = NQ // P  # 8 token chunks
FM = FFN // P  # 16 ffn chunks
SCALE = HD ** -0.5
EPS = 1e-5

INPUT_NAMES = [
    "drug", "prot", "wq", "bq", "wk", "bk", "wv", "bv", "wo", "bo",
    "ln1_g", "ln1_b", "ln2_g", "ln2_b", "w1", "b1", "w2", "b2",
]

_CACHE = {}


def _build(flags, use_f32r=True, act_name="Gelu_apprx_tanh"):
    import concourse.bass as bass
    import concourse.bacc as bacc
    import concourse.mybir as mybir
    import concourse.tile as tile
    from concourse.masks import make_identity

    f32 = mybir.dt.float32
    mmdt = mybir.dt.float32r if use_f32r else f32
    AF = mybir.ActivationFunctionType
    OP = mybir.AluOpType

    (has_bq, has_bk, has_bv, has_bo, has_b1, has_b2,
     has_g1, has_be1, has_g2, has_be2) = flags

    nc = bacc.Bacc(None)

    dr = {}
    shapes = {
        "drug": [NQ, D], "prot": [NK, D],
        "wq": [D, D], "wk": [D, D], "wv": [D, D], "wo": [D, D],
        "bq": [D], "bk": [D], "bv": [D], "bo": [D],
        "ln1_g": [D], "ln1_b": [D], "ln2_g": [D], "ln2_b": [D],
        "w1": [D, FFN], "b1": [FFN], "w2": [FFN, D], "b2": [D],
    }
    mm_input_names = ("wq", "wk", "wv", "wo", "w1", "w2")
    for name in INPUT_NAMES:
        dt_in = mmdt if name in mm_input_names else f32
        dr[name] = nc.dram_tensor(name, shapes[name], dt_in, kind="ExternalInput")
    out_dram = nc.dram_tensor("out", [NQ, D], f32, kind="ExternalOutput")

    def bcast_dram(ap1d, parts):
        # DMA-broadcast a 1-D DRAM vector across `parts` partitions
        return bass.AP(tensor=ap1d.tensor, offset=ap1d.offset,
                       ap=[[0, parts]] + [list(x) for x in ap1d.ap])

    with tile.TileContext(nc) as tc:
        pool = lambda nm, n=1, space="SBUF", side=None: tc.alloc_tile_pool(
            name=nm, bufs=n, space=space, side=side)

        # ---------- constants (whole kernel) ----------
        cn = pool("cn", side="left")
        ident = cn.tile([P, P], f32)
        make_identity(nc, ident)

        bq_col = bk_col = bv_bc = bo_bc = b1_col = b2_bc = None
        g1_bc = be1_bc = g2_bc = be2_bc = None
        if has_bq:
            bq_col = cn.tile([P, KD], f32, tag="bq")
            nc.sync.dma_start(bq_col, dr["bq"][:].rearrange("(ko p) -> p ko", p=P))
        if has_bk:
            bk_col = cn.tile([P, KD], f32, tag="bk")
            nc.sync.dma_start(bk_col, dr["bk"][:].rearrange("(ko p) -> p ko", p=P))
        if has_bv:
            bv_bc = cn.tile([P, D], f32, tag="bv")
            nc.sync.dma_start(bv_bc, bcast_dram(dr["bv"][:], P))
        if has_bo:
            bo_bc = cn.tile([P, D], f32, tag="bo")
            nc.sync.dma_start(bo_bc, bcast_dram(dr["bo"][:], P))
        if has_b2:
            b2_bc = cn.tile([P, D], f32, tag="b2")
            nc.sync.dma_start(b2_bc, bcast_dram(dr["b2"][:], P))
        if has_b1:
            b1_col = cn.tile([P, FM], f32, tag="b1")
            nc.sync.dma_start(b1_col, dr["b1"][:].rearrange("(ko p) -> p ko", p=P))
        if has_g1:
            g1_bc = cn.tile([P, D], f32, tag="g1")
            nc.sync.dma_start(g1_bc, bcast_dram(dr["ln1_g"][:], P))
        if has_be1:
            be1_bc = cn.tile([P, D], f32, tag="be1")
            nc.sync.dma_start(be1_bc, bcast_dram(dr["ln1_b"][:], P))
        if has_g2:
            g2_bc = cn.tile([P, D], f32, tag="g2")
            nc.sync.dma_start(g2_bc, bcast_dram(dr["ln2_g"][:], P))
        if has_be2:
            be2_bc = cn.tile([P, D], f32, tag="be2")
            nc.sync.dma_start(be2_bc, bcast_dram(dr["ln2_b"][:], P))

        eps_t = cn.tile([P, 1], f32, tag="eps")
        nc.vector.memset(eps_t, EPS)
        zero_t = cn.tile([P, 1], f32, tag="zero")
        nc.vector.memset(zero_t, 0.0)
        ones_h = cn.tile([P, H, 1], f32, tag="ones_h")
        nc.vector.memset(ones_h, 1.0)
        # ones row at partition 64, f32r, lhsT of the K=1 broadcast matmul
        onesf = cn.tile([HD + 1, HD], f32, tag="onesf")
        nc.vector.memset(onesf, 1.0)
        ones_r = cn.tile([HD + 1, HD], mmdt, tag="ones_r")
        nc.vector.tensor_copy(ones_r, onesf)
        ident_r = cn.tile([P, P], mmdt, tag="ident_r")
        nc.vector.tensor_copy(ident_r, ident)
        warm_f = cn.tile([P, 512], f32, tag="warm_f")
        nc.vector.memset(warm_f, 0.5)
        warm_src = cn.tile([P, 512], mmdt, tag="warm_src")
        nc.vector.tensor_copy(warm_src, warm_f)

        # tiny long-lived stats tiles
        mv1 = cn.tile([P, QM, 2], f32, tag="mv1")
        mv2 = cn.tile([P, QM, 2], f32, tag="mv2")
        lt1 = cn.tile([P, QM], f32, tag="lt1")
        rstd1 = cn.tile([P, QM], f32, tag="rstd1")
        lt2 = cn.tile([P, QM], f32, tag="lt2")
        rstd2 = cn.tile([P, QM], f32, tag="rstd2")

        def warm_burst(ps_pool, tag, n_mm, bufs=1):
            # dependency-free back-to-back matmuls; >=3.4us of contiguous PE
            # activity releases the HAM clock throttle (K=4/8 -> 8/8)
            wp = ps_pool.tile([P, 512], f32, tag=tag, bufs=bufs)
            for _ in range(n_mm):
                nc.tensor.matmul(wp, lhsT=ident_r, rhs=warm_src,
                                 start=True, stop=True)

        # ---------- phase 1: load + transpose inputs ----------
        pAB = pool("pAB", side="right")
        psA = pool("psA", space="PSUM")  # touch(1) + tp(2) + mm(3) = 6 banks

        drug_nat = pAB.tile([P, QM, D], f32, tag="dn")
        prot_nat = pAB.tile([P, QM, D], f32, tag="pn")
        nc.sync.dma_start(drug_nat, dr["drug"][:].rearrange("(m p) d -> p m d", p=P))
        nc.sync.dma_start(prot_nat, dr["prot"][:].rearrange("(m p) d -> p m d", p=P))
        # QKV weights live with the inputs (all dead after phase 2)
        wq_sb = pAB.tile([P, KD, D], mmdt, tag="wq")
        wk_sb = pAB.tile([P, KD, D], mmdt, tag="wk")
        wv_sb = pAB.tile([P, KD, D], mmdt, tag="wv")
        for w_sb, name in ((wq_sb, "wq"), (wk_sb, "wk"), (wv_sb, "wv")):
            nc.sync.dma_start(
                w_sb, dr[name][:].rearrange("(ko p) n -> p ko n", p=P))

        # absorber matmuls: a transpose's LDW struct only fits ONE sync wait
        # (walrus limit), so let a normal matmul observe each input DMA (and
        # the ident producer) on PE before the transposes run.
        warm_burst(psA, "touch", 10, bufs=1)
        touch = psA.tile([P, 1], f32, tag="touch", bufs=1)
        nc.tensor.matmul(touch, lhsT=ident, rhs=drug_nat[:, 0, 0:1],
                         start=True, stop=True)
        nc.tensor.matmul(touch, lhsT=ident, rhs=prot_nat[:, 0, 0:1],
                         start=True, stop=True)

        drugT = pAB.tile([P, KD, NQ], mmdt, tag="dT")
        protT = pAB.tile([P, KD, NK], mmdt, tag="pT")
        for src, dst in ((drug_nat, drugT), (prot_nat, protT)):
            for m in range(QM):
                for c in range(KD):
                    pt = psA.tile([P, P], f32, tag="tp", bufs=2)
                    nc.tensor.transpose(pt, src[:, m, c * P:(c + 1) * P], ident)
                    nc.vector.tensor_copy(dst[:, c, m * P:(m + 1) * P], pt)

        # ---------- phase 2: QKV projections ----------
        warm_burst(psA, "touch", 10, bufs=1)
        pC = pool("pC", side="left")
        # wo stored per-head at base partition 0 to match ctxT tiles
        wo_sb = pC.tile([HD, H, D], mmdt, tag="wo")
        nc.sync.dma_start(
            wo_sb, dr["wo"][:].rearrange("(h p) n -> p h n", p=HD))
        QT = pC.tile([P, KD, NQ], mmdt, tag="QT")
        KT = pC.tile([P, KD, NK], mmdt, tag="KT")
        for (w_sb, src, dst, bias) in (
            (wq_sb, drugT, QT, bq_col),
            (wk_sb, protT, KT, bk_col),
        ):
            for mo in range(KD):
                for qc in range(2):
                    pq = psA.tile([P, 512], f32, tag="mm", bufs=3)
                    for kd in range(KD):
                        nc.tensor.matmul(
                            pq,
                            lhsT=w_sb[:, kd, mo * P:(mo + 1) * P],
                            rhs=src[:, kd, qc * 512:(qc + 1) * 512],
                            start=(kd == 0), stop=(kd == KD - 1))
                    o = dst[:, mo, qc * 512:(qc + 1) * 512]
                    if bias is not None:
                        nc.vector.tensor_scalar_add(o, pq, bias[:, mo:mo + 1])
                    else:
                        nc.vector.tensor_copy(o, pq)

        # V in natural layout, augmented with ones column per head
        Vaug = pC.tile([P, QM, H, HD + 1], mmdt, tag="Va")
        for m in range(QM):
            pv = psA.tile([P, 512], f32, tag="mm", bufs=3)
            for kd in range(KD):
                nc.tensor.matmul(
                    pv,
                    lhsT=protT[:, kd, m * P:(m + 1) * P],
                    rhs=wv_sb[:, kd, :],
                    start=(kd == 0), stop=(kd == KD - 1))
            o = Vaug[:, m, :, 0:HD]
            pv_v = pv.rearrange("p (h d) -> p h d", h=H)
            if has_bv:
                nc.vector.tensor_add(o, pv_v, bv_bc.rearrange("p (h d) -> p h d", h=H))
            else:
                nc.vector.tensor_copy(o, pv_v)
            nc.vector.tensor_copy(Vaug[:, m, :, HD:HD + 1], ones_h)

        pAB.release()
        psA.release()

        # ---------- phase 3: attention ----------
        pW = pool("pW", side="right")   # w1 prefetched during attention
        w1_sb = pW.tile([P, KD, FFN], mmdt, tag="w1")
        nc.sync.dma_start(w1_sb, dr["w1"][:].rearrange("(ko p) n -> p ko n", p=P))

        pD = pool("pD", side="right")
        pE = pool("pE", side="left")
        psB = pool("psB", space="PSUM")  # sc(2x2) + cx(2x2) = 8 banks

        warm_burst(psB, "sc", 10, bufs=2)

        def normalize(h):
            # softmax normalization of head h: recip sums in place, K=1 PE
            # matmul broadcasts them across the 64 partitions, in-place
            # scale.  Emitted two heads late so every dependency is long
            # resolved and the in-order PE queue never stalls on it.
            ctxU = ctxUs[h]
            with nc.allow_low_precision(reason="f32r softmax denominators"):
                nc.vector.reciprocal(ctxU[HD:HD + 1, :, :],
                                     ctxU[HD:HD + 1, :, :])
            for qc in range(2):
                rbp = psB.tile([HD, 512], f32, tag="rb", bufs=2)
                nc.tensor.matmul(rbp, lhsT=ones_r[HD:HD + 1, :],
                                 rhs=ctxU[HD:HD + 1, qc, :],
                                 start=True, stop=True)
                rbs = pD.tile([HD, 512], f32, tag="rbs", bufs=2)
                nc.vector.tensor_copy(rbs, rbp)
                nc.vector.tensor_mul(ctxU[0:HD, qc, :], ctxU[0:HD, qc, :], rbs)

        ctxUs = []
        for h in range(H):
            kt_c = h // 2
            roff = HD * (h % 2)
            cx = psB.tile([HD + 1, 2, 512], f32, tag="cx", bufs=1)
            for kc in range(QM):
                sc_ps = psB.tile([P, 1024], f32, tag="sc", bufs=2)
                for qc in range(2):
                    nc.tensor.matmul(
                        sc_ps[:, qc * 512:(qc + 1) * 512],
                        lhsT=KT[roff:roff + HD, kt_c, kc * P:(kc + 1) * P],
                        rhs=QT[roff:roff + HD, kt_c, qc * 512:(qc + 1) * 512],
                        start=True, stop=True)
                et = pD.tile([P, 1024], mmdt, tag="et", bufs=3)
                nc.scalar.activation(et, sc_ps, AF.Exp, scale=SCALE)
                for qc in range(2):
                    nc.tensor.matmul(
                        cx[:, qc, :],
                        lhsT=Vaug[:, kc, h, :],
                        rhs=et[:, qc * 512:(qc + 1) * 512],
                        start=(kc == 0), stop=(kc == QM - 1))
            # move unnormalized ctx + sums row to SBUF; frees the PSUM slot
            ctxU = pE.tile([HD + 1, 2, 512], mmdt, tag="ctxU", bufs=H)
            nc.vector.tensor_copy(ctxU, cx)
            ctxUs.append(ctxU)
            if h >= 2:
                normalize(h - 2)
        normalize(H - 2)
        normalize(H - 1)

        pD.release()
        psB.release()

        # ---------- phase 4: attn_out + residual + LN1 ----------
        pF = pool("pF", side="right")
        psC = pool("psC", space="PSUM")  # mm(2) + tp(2) + f1(2x2) = 8 banks

        warm_burst(psC, "mm", 10, bufs=2)
        drug2 = pF.tile([P, QM, D], f32, tag="dn2")
        nc.sync.dma_start(drug2, dr["drug"][:].rearrange("(m p) d -> p m d", p=P))
        x_nat = pF.tile([P, QM, D], f32, tag="xn")
        for qm in range(QM):
            pa = psC.tile([P, 512], f32, tag="mm", bufs=2)
            for h in range(H):
                nc.tensor.matmul(
                    pa,
                    lhsT=ctxUs[h][0:HD, :, :].rearrange("p a b -> p (a b)")[
                        :, qm * P:(qm + 1) * P],
                    rhs=wo_sb[:, h, :],
                    start=(h == 0), stop=(h == H - 1))
            t = x_nat[:, qm, :]
            nc.vector.tensor_add(t, pa, drug2[:, qm, :])
            if has_bo:
                nc.vector.tensor_add(t, t, bo_bc)
            st = pF.tile([P, nc.vector.BN_STATS_DIM], f32, tag="st", bufs=2)
            nc.vector.bn_stats(st, t)
            nc.vector.bn_aggr(mv1[:, qm, :], st)
            nc.scalar.activation(lt1[:, qm:qm + 1], mv1[:, qm, 1:2],
                                 AF.Ln, bias=eps_t)
            nc.scalar.activation(rstd1[:, qm:qm + 1], lt1[:, qm:qm + 1],
                                 AF.Exp, scale=-0.5)
            nc.vector.tensor_scalar(t, t, mv1[:, qm, 0:1], rstd1[:, qm:qm + 1],
                                    OP.subtract, OP.mult)
            if has_g1:
                nc.vector.tensor_mul(t, t, g1_bc)
            if has_be1:
                nc.vector.tensor_add(t, t, be1_bc)

        pE.release()  # ctxU dead
        pC.release()  # QT/KT/Vaug/wo dead

        # ---------- phase 5+6: x transpose + FFN ----------
        pG = pool("pG", side="left")
        w2_sb = pG.tile([P, FM, D], mmdt, tag="w2")
        nc.sync.dma_start(w2_sb, dr["w2"][:].rearrange("(ko p) n -> p ko n", p=P))

        xT = pG.tile([P, KD, NQ], mmdt, tag="xT")
        for m in range(QM):
            for c in range(KD):
                pt = psC.tile([P, P], f32, tag="tp", bufs=2)
                nc.tensor.transpose(pt, x_nat[:, m, c * P:(c + 1) * P], ident)
                nc.vector.tensor_copy(xT[:, c, m * P:(m + 1) * P], pt)

        out_v = out_dram[:].rearrange("(m p) d -> p m d", p=P)
        x2 = pF.tile([P, QM, D], f32, tag="dn2")  # reuse drug2's slot
        for qh in range(2):  # FFN in two query halves to halve h1 residency
            h1T = pG.tile([P, FM, 512], mmdt, tag="h1", bufs=1)
            for mo in range(FM):
                pf = psC.tile([P, 512], f32, tag="f1", bufs=2)
                for kd in range(KD):
                    nc.tensor.matmul(
                        pf,
                        lhsT=w1_sb[:, kd, mo * P:(mo + 1) * P],
                        rhs=xT[:, kd, qh * 512:(qh + 1) * 512],
                        start=(kd == 0), stop=(kd == KD - 1))
                nc.scalar.activation(
                    h1T[:, mo, :], pf, getattr(AF, act_name),
                    bias=(b1_col[:, mo:mo + 1] if has_b1 else zero_t))
            for qj in range(4):
                qm = qh * 4 + qj
                pf2 = psC.tile([P, 512], f32, tag="mm", bufs=2)
                for kc in range(FM):
                    nc.tensor.matmul(
                        pf2,
                        lhsT=h1T[:, kc, qj * P:(qj + 1) * P],
                        rhs=w2_sb[:, kc, :],
                        start=(kc == 0), stop=(kc == FM - 1))
                t = x2[:, qm, :]
                nc.vector.tensor_add(t, pf2, x_nat[:, qm, :])
                if has_b2:
                    nc.vector.tensor_add(t, t, b2_bc)
                st = pF.tile([P, nc.vector.BN_STATS_DIM], f32, tag="st", bufs=2)
                nc.vector.bn_stats(st, t)
                nc.vector.bn_aggr(mv2[:, qm, :], st)
                nc.scalar.activation(lt2[:, qm:qm + 1], mv2[:, qm, 1:2],
                                     AF.Ln, bias=eps_t)
                nc.scalar.activation(rstd2[:, qm:qm + 1], lt2[:, qm:qm + 1],
                                     AF.Exp, scale=-0.5)
                ob = pG.tile([P, D], f32, tag="ob", bufs=3)
                nc.vector.tensor_scalar(ob, t, mv2[:, qm, 0:1],
                                        rstd2[:, qm:qm + 1],
                                        OP.subtract, OP.mult)
                if has_g2:
                    nc.vector.tensor_mul(ob, ob, g2_bc)
                if has_be2:
                    nc.vector.tensor_add(ob, ob, be2_bc)
                nc.sync.dma_start(out_v[:, qm, :], ob)

        pF.release()
        pW.release()
        pG.release()
        psC.release()
        cn.release()  # left stack: pG popped first, then cn

    nc.finalize()
    return nc


def _flags_from_inputs(inputs):
    def nz(name):
        return bool(np.any(inputs[name] != 0.0))

    return (
        nz("bq"), nz("bk"), nz("bv"), nz("bo"), nz("b1"), nz("b2"),
        bool(np.any(inputs["ln1_g"] != 1.0)), nz("ln1_b"),
        bool(np.any(inputs["ln2_g"] != 1.0)), nz("ln2_b"),
    )


def build_nc(inputs, use_f32r=True, act_name="Gelu_apprx_tanh"):
    flags = _flags_from_inputs(inputs)
    key = (flags, use_f32r, act_name)
    if key not in _CACHE:
        _CACHE[key] = _build(flags, use_f32r=use_f32r, act_name=act_name)
    return _CACHE[key]


def kernel(**inputs):
    from concourse.bass_utils import run_bass_kernel_spmd

    inputs = {k: np.ascontiguousarray(np.asarray(v, dtype=np.float32))
              for k, v in inputs.items()}
    nc = build_nc(inputs)
    in_maps = []
    for b in range(B):
        m = {}
        for name in INPUT_NAMES:
            v = inputs[name]
            m[name] = v[b] if name in ("drug", "prot") else v
        in_maps.append(m)
    res = run_bass_kernel_spmd(nc, in_maps, list(range(B)))
    out = np.stack([res.results[i]["out"] for i in range(B)], axis=0)
    return out.astype(np.float32)


# revision 42
# speedup vs baseline: 1.2523x; 1.0512x over previous
"""Trainium2 Bass kernel for AdvancedDualTargetPredictor (cross-attention
transformer block).

Sharding: pure data-parallel over batch B=8 across the 8 NeuronCores.
Each core runs one batch element end-to-end; no collectives.

Per-core dataflow (token dim q/k = 1024, model dim D = 512, H = 8 heads,
head dim 64, FFN = 2048):
  1. PE-transpose drug/prot -> drugT/protT  [D, tok] layout
  2. QT = wq.T @ drugT, KT = wk.T @ protT   [D, tok]
     V  = protT.T @ wv                       [tok, D] (natural), augmented
          with a ones column per head for softmax row sums
  3. per head: scoresT = K @ Q.T  (PSUM), exp via ACT (scale=1/8 fused),
     ctx^T/sums = [V_h | 1].T @ ET  accumulated over k chunks, copied to
     SBUF unnormalized; after the loop all heads are normalized in a
     batch: reciprocal of the sums row, K=1 PE matmul broadcast across
     64 partitions, in-place scale.  (No DMA in the critical chain and
     double-buffered PSUM everywhere, so the PE never idles long enough
     for the HAM clock throttle to kick in.)
  4. attn_out = ctxT.T @ wo (+bo), residual + LN1 (bn_stats, natural layout)
  5. PE-transpose x -> xT; h1T = w1.T @ xT with fused gelu(+b1) on ACT
     (w1 is prefetched during attention)
  6. ffn = h1T.T @ w2 (+b2), residual + LN2 -> out
All matmuls run as float32r (full PE rate for free dim >= 256); every
producer feeding a matmul writes float32r (rounded) per the BIR verifier.
Pools are allocated/released per phase and per SBUF side: reservations are
static per pool lifetime, so phase scoping keeps peak usage in budget.
"""

import numpy as np

B, NQ, NK, D, H = 8, 1024, 1024, 512, 8
HD = D // H  # 64
FFN = 4 * D  # 2048
P = 128
KD = D // P  # 4 chunks of the model dim
QM = NQ // P  # 8 token chunks
FM = FFN // P  # 16 ffn chunks
SCALE = HD ** -0.5
EPS = 1e-5

INPUT_NAMES = [
    "drug", "prot", "wq", "bq", "wk", "bk", "wv", "bv", "wo", "bo",
    "ln1_g", "ln1_b", "ln2_g", "ln2_b", "w1", "b1", "w2", "b2",
]

_CACHE = {}


def _build(flags, use_f32r=True, act_name="Gelu_apprx_tanh"):
    import concourse.bass as bass
    import concourse.bacc as bacc
    import concourse.mybir as mybir
    import concourse.tile as tile
    from concourse.masks import make_identity

    f32 = mybir.dt.float32
    mmdt = mybir.dt.float32r if use_f32r else f32
    AF = mybir.ActivationFunctionType
    OP = mybir.AluOpType

    (has_bq, has_bk, has_bv, has_bo, has_b1, has_b2,
     has_g1, has_be1, has_g2, has_be2) = flags

    nc = bacc.Bacc(None)

    dr = {}
    shapes = {
        "drug": [NQ, D], "prot": [NK, D],
        "wq": [D, D], "wk": [D, D], "wv": [D, D], "wo": [D, D],
        "bq": [D], "bk": [D], "bv": [D], "bo": [D],
        "ln1_g": [D], "ln1_b": [D], "ln2_g": [D], "ln2_b": [D],
        "w1": [D, FFN], "b1": [FFN], "w2": [FFN, D], "b2": [D],
    }
    mm_input_names = ("wq", "wk", "wv", "wo", "w1", "w2")
    for name in INPUT_NAMES:
        dt_in = mmdt if name in mm_input_names else f32
        dr[name] = nc.dram_tensor(name, shapes[name], dt_in, kind="ExternalInput")
    out_dram = nc.dram_tensor("out", [NQ, D], f32, kind="ExternalOutput")

    def bcast_dram(ap1d, parts):
        # DMA-broadcast a 1-D DRAM vector across `parts` partitions
        return bass.AP(tensor=ap1d.tensor, offset=ap1d.offset,
                       ap=[[0, parts]] + [list(x) for x in ap1d.ap])

    with tile.TileContext(nc) as tc:
        pool = lambda nm, n=1, space="SBUF", side=None: tc.alloc_tile_pool(
            name=nm, bufs=n, space=space, side=side)

        # ---------- constants (whole kernel) ----------
        cn = pool("cn", side="left")
        ident = cn.tile([P, P], f32)
        make_identity(nc, ident)

        bq_col = bk_col = bv_bc = bo_bc = b1_col = b2_bc = None
        g1_bc = be1_bc = g2_bc = be2_bc = None
        if has_bq:
            bq_col = cn.tile([P, KD], f32, tag="bq")
            nc.sync.dma_start(bq_col, dr["bq"][:].rearrange("(ko p) -> p ko", p=P))
        if has_bk:
            bk_col = cn.tile([P, KD], f32, tag="bk")
            nc.sync.dma_start(bk_col, dr["bk"][:].rearrange("(ko p) -> p ko", p=P))
        if has_bv:
            bv_bc = cn.tile([P, D], f32, tag="bv")
            nc.sync.dma_start(bv_bc, bcast_dram(dr["bv"][:], P))
        if has_bo:
            bo_bc = cn.tile([P, D], f32, tag="bo")
            nc.sync.dma_start(bo_bc, bcast_dram(dr["bo"][:], P))
        if has_b2:
            b2_bc = cn.tile([P, D], f32, tag="b2")
            nc.sync.dma_start(b2_bc, bcast_dram(dr["b2"][:], P))
        if has_b1:
            b1_col = cn.tile([P, FM], f32, tag="b1")
            nc.sync.dma_start(b1_col, dr["b1"][:].rearrange("(ko p) -> p ko", p=P))
        if has_g1:
            g1_bc = cn.tile([P, D], f32, tag="g1")
            nc.sync.dma_start(g1_bc, bcast_dram(dr["ln1_g"][:], P))
        if has_be1:
            be1_bc = cn.tile([P, D], f32, tag="be1")
            nc.sync.dma_start(be1_bc, bcast_dram(dr["ln1_b"][:], P))
        if has_g2:
            g2_bc = cn.tile([P, D], f32, tag="g2")
            nc.sync.dma_start(g2_bc, bcast_dram(dr["ln2_g"][:], P))
        if has_be2:
            be2_bc = cn.tile([P, D], f32, tag="be2")
            nc.sync.dma_start(be2_bc, bcast_dram(dr["ln2_b"][:], P))

        eps_t = cn.tile([P, 1], f32, tag="eps")
        nc.vector.memset(eps_t, EPS)
        zero_t = cn.tile([P, 1], f32, tag="zero")
        nc.vector.memset(zero_t, 0.0)
        ones_h = cn.tile([P, H, 1], f32, tag="ones_h")
        nc.vector.memset(ones_h, 1.0)
        # ones row at partition 64, f32r, lhsT of the K=1 broadcast matmul
        onesf = cn.tile([HD + 1, HD], f32, tag="onesf")
        nc.vector.memset(onesf, 1.0)
        ones_r = cn.tile([HD + 1, HD], mmdt, tag="ones_r")
        nc.vector.tensor_copy(ones_r, onesf)
        ident_r = cn.tile([P, P], mmdt, tag="ident_r")
        nc.vector.tensor_copy(ident_r, ident)
        warm_f = cn.tile([P, 512], f32, tag="warm_f")
        nc.vector.memset(warm_f, 0.5)
        warm_src = cn.tile([P, 512], mmdt, tag="warm_src")
        nc.vector.tensor_copy(warm_src, warm_f)

        # tiny long-lived stats tiles
        mv1 = cn.tile([P, QM, 2], f32, tag="mv1")
        mv2 = cn.tile([P, QM, 2], f32, tag="mv2")
        lt1 = cn.tile([P, QM], f32, tag="lt1")
        rstd1 = cn.tile([P, QM], f32, tag="rstd1")
        lt2 = cn.tile([P, QM], f32, tag="lt2")
        rstd2 = cn.tile([P, QM], f32, tag="rstd2")

        def warm_burst(ps_pool, tag, n_mm, bufs=1):
            # dependency-free back-to-back matmuls; >=3.4us of contiguous PE
            # activity releases the HAM clock throttle (K=4/8 -> 8/8)
            wp = ps_pool.tile([P, 512], f32, tag=tag, bufs=bufs)
            for _ in range(n_mm):
                nc.tensor.matmul(wp, lhsT=ident_r, rhs=warm_src,
                                 start=True, stop=True)

        # ---------- phase 1: load + transpose inputs ----------
        pAB = pool("pAB", side="right")
        psA = pool("psA", space="PSUM")  # touch(1) + tp(2) + mm(3) = 6 banks

        drug_nat = pAB.tile([P, QM, D], f32, tag="dn")
        prot_nat = pAB.tile([P, QM, D], f32, tag="pn")
        nc.sync.dma_start(drug_nat, dr["drug"][:].rearrange("(m p) d -> p m d", p=P))
        nc.sync.dma_start(prot_nat, dr["prot"][:].rearrange("(m p) d -> p m d", p=P))
        # QKV weights live with the inputs (all dead after phase 2)
        wq_sb = pAB.tile([P, KD, D], mmdt, tag="wq")
        wk_sb = pAB.tile([P, KD, D], mmdt, tag="wk")
        wv_sb = pAB.tile([P, KD, D], mmdt, tag="wv")
        for w_sb, name in ((wq_sb, "wq"), (wk_sb, "wk"), (wv_sb, "wv")):
            nc.sync.dma_start(
                w_sb, dr[name][:].rearrange("(ko p) n -> p ko n", p=P))

        # absorber matmuls: a transpose's LDW struct only fits ONE sync wait
        # (walrus limit), so let a normal matmul observe each input DMA (and
        # the ident producer) on PE before the transposes run.
        warm_burst(psA, "touch", 10, bufs=1)
        touch = psA.tile([P, 1], f32, tag="touch", bufs=1)
        nc.tensor.matmul(touch, lhsT=ident, rhs=drug_nat[:, 0, 0:1],
                         start=True, stop=True)
        nc.tensor.matmul(touch, lhsT=ident, rhs=prot_nat[:, 0, 0:1],
                         start=True, stop=True)

        drugT = pAB.tile([P, KD, NQ], mmdt, tag="dT")
        protT = pAB.tile([P, KD, NK], mmdt, tag="pT")
        for src, dst in ((drug_nat, drugT), (prot_nat, protT)):
            for m in range(QM):
                for c in range(KD):
                    pt = psA.tile([P, P], f32, tag="tp", bufs=2)
                    nc.tensor.transpose(pt, src[:, m, c * P:(c + 1) * P], ident)
                    nc.vector.tensor_copy(dst[:, c, m * P:(m + 1) * P], pt)

        # ---------- phase 2: QKV projections ----------
        warm_burst(psA, "touch", 10, bufs=1)
        pC = pool("pC", side="left")
        # wo stored per-head at base partition 0 to match ctxT tiles
        wo_sb = pC.tile([HD, H, D], mmdt, tag="wo")
        nc.sync.dma_start(
            wo_sb, dr["wo"][:].rearrange("(h p) n -> p h n", p=HD))
        QT = pC.tile([P, KD, NQ], mmdt, tag="QT")
        KT = pC.tile([P, KD, NK], mmdt, tag="KT")
        for (w_sb, src, dst, bias) in (
            (wq_sb, drugT, QT, bq_col),
            (wk_sb, protT, KT, bk_col),
        ):
            for mo in range(KD):
                for qc in range(2):
                    pq = psA.tile([P, 512], f32, tag="mm", bufs=3)
                    for kd in range(KD):
                        nc.tensor.matmul(
                            pq,
                            lhsT=w_sb[:, kd, mo * P:(mo + 1) * P],
                            rhs=src[:, kd, qc * 512:(qc + 1) * 512],
                            start=(kd == 0), stop=(kd == KD - 1))
                    o = dst[:, mo, qc * 512:(qc + 1) * 512]
                    if bias is not None:
                        nc.vector.tensor_scalar_add(o, pq, bias[:, mo:mo + 1])
                    else:
                        nc.vector.tensor_copy(o, pq)

        # V in natural layout, augmented with ones column per head
        Vaug = pC.tile([P, QM, H, HD + 1], mmdt, tag="Va")
        for m in range(QM):
            pv = psA.tile([P, 512], f32, tag="mm", bufs=3)
            for kd in range(KD):
                nc.tensor.matmul(
                    pv,
                    lhsT=protT[:, kd, m * P:(m + 1) * P],
                    rhs=wv_sb[:, kd, :],
                    start=(kd == 0), stop=(kd == KD - 1))
            o = Vaug[:, m, :, 0:HD]
            pv_v = pv.rearrange("p (h d) -> p h d", h=H)
            if has_bv:
                nc.vector.tensor_add(o, pv_v, bv_bc.rearrange("p (h d) -> p h d", h=H))
            else:
                nc.vector.tensor_copy(o, pv_v)
            nc.vector.tensor_copy(Vaug[:, m, :, HD:HD + 1], ones_h)

        pAB.release()
        psA.release()

        # ---------- phase 3: attention ----------
        pW = pool("pW", side="right")   # w1 prefetched during attention
        w1_sb = pW.tile([P, KD, FFN], mmdt, tag="w1")
        nc.sync.dma_start(w1_sb, dr["w1"][:].rearrange("(ko p) n -> p ko n", p=P))

        pD = pool("pD", side="right")
        pE = pool("pE", side="left")
        psB = pool("psB", space="PSUM")  # sc(2x2) + cx(2x2) = 8 banks

        warm_burst(psB, "sc", 10, bufs=2)

        def normalize(h):
            # softmax normalization of head h: recip sums in place, K=1 PE
            # matmul broadcasts them across the 64 partitions, in-place
            # scale.  Emitted two heads late so every dependency is long
            # resolved and the in-order PE queue never stalls on it.
            ctxU = ctxUs[h]
            with nc.allow_low_precision(reason="f32r softmax denominators"):
                nc.vector.reciprocal(ctxU[HD:HD + 1, :, :],
                                     ctxU[HD:HD + 1, :, :])
            for qc in range(2):
                rbp = psB.tile([HD, 512], f32, tag="cx", bufs=2)
                nc.tensor.matmul(rbp, lhsT=ones_r[HD:HD + 1, :],
                                 rhs=ctxU[HD:HD + 1, qc, :],
                                 start=True, stop=True)
                rbs = pD.tile([HD, 512], f32, tag="rbs", bufs=2)
                nc.vector.tensor_copy(rbs, rbp)
                nc.vector.tensor_mul(ctxU[0:HD, qc, :], ctxU[0:HD, qc, :], rbs)

        ctxUs = []
        for h in range(H):
            kt_c = h // 2
            roff = HD * (h % 2)
            cx = psB.tile([HD + 1, 2, 512], f32, tag="cx", bufs=2)
            for kc in range(QM):
                sc_ps = psB.tile([P, 1024], f32, tag="sc", bufs=2)
                for qc in range(2):
                    nc.tensor.matmul(
                        sc_ps[:, qc * 512:(qc + 1) * 512],
                        lhsT=KT[roff:roff + HD, kt_c, kc * P:(kc + 1) * P],
                        rhs=QT[roff:roff + HD, kt_c, qc * 512:(qc + 1) * 512],
                        start=True, stop=True)
                et = pD.tile([P, 1024], mmdt, tag="et", bufs=3)
                nc.scalar.activation(et, sc_ps, AF.Exp, scale=SCALE)
                for qc in range(2):
                    nc.tensor.matmul(
                        cx[:, qc, :],
                        lhsT=Vaug[:, kc, h, :],
                        rhs=et[:, qc * 512:(qc + 1) * 512],
                        start=(kc == 0), stop=(kc == QM - 1))
            # move unnormalized ctx + sums row to SBUF; frees the PSUM slot
            ctxU = pE.tile([HD + 1, 2, 512], mmdt, tag="ctxU", bufs=H)
            nc.vector.tensor_copy(ctxU, cx)
            ctxUs.append(ctxU)
            if h >= 2:
                normalize(h - 2)
            if h in (2, 4, 6):
                warm_burst(psB, "sc", 10, bufs=2)
        normalize(H - 2)
        normalize(H - 1)

        pD.release()
        psB.release()

        # ---------- phase 4: attn_out + residual + LN1 ----------
        pF = pool("pF", side="right")
        psC = pool("psC", space="PSUM")  # mm(2) + tp(2) + f1(2x2) = 8 banks

        warm_burst(psC, "mm", 10, bufs=2)
        drug2 = pF.tile([P, QM, D], f32, tag="dn2")
        nc.sync.dma_start(drug2, dr["drug"][:].rearrange("(m p) d -> p m d", p=P))
        x_nat = pF.tile([P, QM, D], f32, tag="xn")
        for qm in range(QM):
            pa = psC.tile([P, 512], f32, tag="mm", bufs=2)
            for h in range(H):
                nc.tensor.matmul(
                    pa,
                    lhsT=ctxUs[h][0:HD, :, :].rearrange("p a b -> p (a b)")[
                        :, qm * P:(qm + 1) * P],
                    rhs=wo_sb[:, h, :],
                    start=(h == 0), stop=(h == H - 1))
            t = x_nat[:, qm, :]
            nc.vector.tensor_add(t, pa, drug2[:, qm, :])
            if has_bo:
                nc.vector.tensor_add(t, t, bo_bc)
            st = pF.tile([P, nc.vector.BN_STATS_DIM], f32, tag="st", bufs=2)
            nc.vector.bn_stats(st, t)
            nc.vector.bn_aggr(mv1[:, qm, :], st)
            nc.scalar.activation(lt1[:, qm:qm + 1], mv1[:, qm, 1:2],
                                 AF.Ln, bias=eps_t)
            nc.scalar.activation(rstd1[:, qm:qm + 1], lt1[:, qm:qm + 1],
                                 AF.Exp, scale=-0.5)
            nc.vector.tensor_scalar(t, t, mv1[:, qm, 0:1], rstd1[:, qm:qm + 1],
                                    OP.subtract, OP.mult)
            if has_g1:
                nc.vector.tensor_mul(t, t, g1_bc)
            if has_be1:
                nc.vector.tensor_add(t, t, be1_bc)

        pE.release()  # ctxU dead
        pC.release()  # QT/KT/Vaug/wo dead

        # ---------- phase 5+6: x transpose + FFN ----------
        pG = pool("pG", side="left")
        w2_sb = pG.tile([P, FM, D], mmdt, tag="w2")
        nc.sync.dma_start(w2_sb, dr["w2"][:].rearrange("(ko p) n -> p ko n", p=P))

        xT = pG.tile([P, KD, NQ], mmdt, tag="xT")
        for m in range(QM):
            for c in range(KD):
                pt = psC.tile([P, P], f32, tag="tp", bufs=2)
                nc.tensor.transpose(pt, x_nat[:, m, c * P:(c + 1) * P], ident)
                nc.vector.tensor_copy(xT[:, c, m * P:(m + 1) * P], pt)

        out_v = out_dram[:].rearrange("(m p) d -> p m d", p=P)
        x2 = pF.tile([P, QM, D], f32, tag="dn2")  # reuse drug2's slot
        for qh in range(2):  # FFN in two query halves to halve h1 residency
            h1T = pG.tile([P, FM, 512], mmdt, tag="h1", bufs=1)
            for mo in range(FM):
                pf = psC.tile([P, 512], f32, tag="f1", bufs=2)
                for kd in range(KD):
                    nc.tensor.matmul(
                        pf,
                        lhsT=w1_sb[:, kd, mo * P:(mo + 1) * P],
                        rhs=xT[:, kd, qh * 512:(qh + 1) * 512],
                        start=(kd == 0), stop=(kd == KD - 1))
                nc.scalar.activation(
                    h1T[:, mo, :], pf, getattr(AF, act_name),
                    bias=(b1_col[:, mo:mo + 1] if has_b1 else zero_t))
            for qj in range(4):
                qm = qh * 4 + qj
                pf2 = psC.tile([P, 512], f32, tag="mm", bufs=2)
                for kc in range(FM):
                    nc.tensor.matmul(
                        pf2,
                        lhsT=h1T[:, kc, qj * P:(qj + 1) * P],
                        rhs=w2_sb[:, kc, :],
                        start=(kc == 0), stop=(kc == FM - 1))
                t = x2[:, qm, :]
                nc.vector.tensor_add(t, pf2, x_nat[:, qm, :])
                if has_b2:
                    nc.vector.tensor_add(t, t, b2_bc)
                st = pF.tile([P, nc.vector.BN_STATS_DIM], f32, tag="st", bufs=2)
                nc.vector.bn_stats(st, t)
                nc.vector.bn_aggr(mv2[:, qm, :], st)
                nc.scalar.activation(lt2[:, qm:qm + 1], mv2[:, qm, 1:2],
                                     AF.Ln, bias=eps_t)
                nc.scalar.activation(rstd2[:, qm:qm + 1], lt2[:, qm:qm + 1],
                                     AF.Exp, scale=-0.5)
                ob = pG.tile([P, D], f32, tag="ob", bufs=3)
                nc.vector.tensor_scalar(ob, t, mv2[:, qm, 0:1],
                                        rstd2[:, qm:qm + 1],
                                        OP.subtract, OP.mult)
                if has_g2:
                    nc.vector.tensor_mul(ob, ob, g2_bc)
                if has_be2:
                    nc.vector.tensor_add(ob, ob, be2_bc)
                nc.sync.dma_start(out_v[:, qm, :], ob)

        pF.release()
        pW.release()
        pG.release()
        psC.release()
        cn.release()  # left stack: pG popped first, then cn

    nc.finalize()
    return nc


def _flags_from_inputs(inputs):
    def nz(name):
        return bool(np.any(inputs[name] != 0.0))

    return (
        nz("bq"), nz("bk"), nz("bv"), nz("bo"), nz("b1"), nz("b2"),
        bool(np.any(inputs["ln1_g"] != 1.0)), nz("ln1_b"),
        bool(np.any(inputs["ln2_g"] != 1.0)), nz("ln2_b"),
    )


def build_nc(inputs, use_f32r=True, act_name="Gelu_apprx_tanh"):
    flags = _flags_from_inputs(inputs)
    key = (flags, use_f32r, act_name)
    if key not in _CACHE:
        _CACHE[key] = _build(flags, use_f32r=use_f32r, act_name=act_name)
    return _CACHE[key]


def kernel(**inputs):
    from concourse.bass_utils import run_bass_kernel_spmd

    inputs = {k: np.ascontiguousarray(np.asarray(v, dtype=np.float32))
              for k, v in inputs.items()}
    nc = build_nc(inputs)
    in_maps = []
    for b in range(B):
        m = {}
        for name in INPUT_NAMES:
            v = inputs[name]
            m[name] = v[b] if name in ("drug", "prot") else v
        in_maps.append(m)
    res = run_bass_kernel_spmd(nc, in_maps, list(range(B)))
    out = np.stack([res.results[i]["out"] for i in range(B)], axis=0)
    return out.astype(np.float32)
